# Initial kernel scaffold
#
"""CrossAttention kernel for 8 TRN2 NeuronCores.

Reference computation (B=2, Lq=4096, Lkv=1024, query_dim=512, cross_dim=768,
heads=8, dim_head=64, inner=512):
    q = hs @ Wq; k = enc @ Wk; v = enc @ Wv          (per batch)
    attn = softmax(q_h @ k_h^T * scale) per head
    out = concat_h(attn @ v_h) @ Wo + bo

Sharding: 8 cores = 2 batches x 4 query-slices of 1024 queries.  Each core
computes its full slice of the output (all heads), so outputs are disjoint
and no collective is needed.

Per-core dataflow (all matmuls bf16 operands, fp32 PSUM accumulate):
  - host passes hs-slice and encoder transposed (hsT [512,1024], encT
    [768,1024]) already cast to bf16, weights in bf16
  - qT = Wq^T-weighted hsT   -> [inner=512, q=1024]  (heads along partitions)
  - kT likewise              -> [inner=512, kv=1024]
  - v natural                -> [kv=1024, slots]  slot h = 128 cols holding
        v_h (64) + a ones column + zero padding, arranged so the AV matmul
        output lands partition-aligned with head h's rows of outT and the
        softmax denominator (sum_kv exp) falls out of the same matmul.
  - scoresT_h = k_h qT_h     -> [kv, q] (kv on partitions; head pairs use
        base-partition row tiling of the 128x128 PE array)
  - expT = exp(scale * scoresT) on ScalarE, bf16 out (no max-subtraction:
        |scaled scores| < ~3)
  - outT_unnorm_h = v_slot^T @ expT accumulated over kv chunks (PSUM),
        one row of which is the softmax denominator
  - normalize: reciprocal (DVE) + PE ones-column broadcast matmul + multiply
  - final = outT^T @ Wo + bo -> [1024, 512], DMA out per 128-row tile

Program order is pipelined for the Tile scheduler: k/v/q projections are
emitted ahead of the attention blocks that consume them, exp(t) is emitted
before AV(t-1) so the PE never waits in-order on ScalarE, and the final
projection m-tiles are interleaved between the last attention blocks.
"""

import sys

if "/opt/trn_rl_repo" not in sys.path:
    sys.path.insert(0, "/opt/trn_rl_repo")

import numpy as np

B, LQ, LKV = 2, 4096, 1024
QD, CD = 512, 768
H, DH = 8, 64
INNER = H * DH  # 512
SCALE = DH ** -0.5
NCORES = 8
QSH = LQ // 4  # 1024 queries per core
P = 128

_CACHE: dict = {}
LAST_RESULTS = None  # test harness introspection (exec_time_ns etc.)

# schedule-tuning knobs (sweepable from bench tooling)
CFG = {
    "W1": 12,       # warmup matmuls bridging the input-DMA head
    "B0_PRE": 8,    # block (0,0) pre-loop extra pops
    "B0_PER": 9,    # block (0,0) per-iter extra pops
    "BK_PRE": 2,    # later n=0 blocks pre-loop pops
    "BK_PER": 2,    # later n=0 blocks per-iter pops
    "B21_PER": 1,   # block (2,1) per-iter pops
    "B31_PER": 1,   # block (3,1) per-iter pops
}


def _build_nc():
    from contextlib import ExitStack

    import concourse.bass as bass
    import concourse.tile as tile
    from concourse import bacc, mybir

    f32 = mybir.dt.float32
    bf16 = mybir.dt.bfloat16
    Exp = mybir.ActivationFunctionType.Exp

    nc = bacc.Bacc(trn_type="TRN2")

    hsT_d = nc.declare_dram_parameter("hsT", [QD, QSH], bf16, isOutput=False)
    encT_d = nc.declare_dram_parameter("encT", [CD, LKV], bf16, isOutput=False)
    wq_d = nc.declare_dram_parameter("wq", [QD, INNER], bf16, isOutput=False)
    wk_d = nc.declare_dram_parameter("wk", [CD, INNER], bf16, isOutput=False)
    wv_d = nc.declare_dram_parameter("wv", [CD, INNER], bf16, isOutput=False)
    wo_d = nc.declare_dram_parameter("wo", [INNER, QD], bf16, isOutput=False)
    bo_d = nc.declare_dram_parameter("bo", [1, QD], f32, isOutput=False)
    out_d = nc.declare_dram_parameter("out", [QSH, QD], f32, isOutput=True)

    KC_Q = QD // P   # 4 contraction chunks for q projection
    KC_KV = CD // P  # 6 for k/v projections
    AT = INNER // P  # 4 inner tiles (2 heads each)
    NT = LKV // P    # 8 kv chunks
    QN = QSH // 512  # 2 q slices of 512

    with ExitStack() as ctx:
        tc = ctx.enter_context(tile.TileContext(nc))
        const = ctx.enter_context(tc.tile_pool(name="const", bufs=1))
        acts = ctx.enter_context(tc.tile_pool(name="acts", bufs=1))
        expp = ctx.enter_context(tc.tile_pool(name="expp", bufs=4))
        outp = ctx.enter_context(tc.tile_pool(name="outp", bufs=4))
        small = ctx.enter_context(tc.tile_pool(name="small", bufs=6))
        psA = ctx.enter_context(tc.tile_pool(name="psA", bufs=4, space="PSUM"))
        psS = ctx.enter_context(tc.tile_pool(name="psS", bufs=2, space="PSUM"))
        drp = ctx.enter_context(tc.tile_pool(name="drp", bufs=4, space="DRAM"))

        # ---- input DMA, ordered by first use: the q projections (hsT+wq)
        # run during the PE warmup window, then kT (encT+wk), then v (wv);
        # the second encT half only gates scores t>=4 of the first block
        hsT_sb = acts.tile([P, KC_Q, QSH], bf16)
        nc.sync.dma_start(hsT_sb[:], hsT_d.rearrange("(c p) n -> p c n", p=P))
        wq_sb = const.tile([P, KC_Q, INNER], bf16)
        nc.sync.dma_start(wq_sb[:], wq_d.rearrange("(c p) n -> p c n", p=P))
        encT_sb = acts.tile([P, KC_KV, LKV], bf16)
        encT_r = encT_d.rearrange("(c p) n -> p c n", p=P)
        nc.sync.dma_start(encT_sb[:, :, 0:512], encT_r[:, :, 0:512])
        wk_sb = const.tile([P, KC_KV, INNER], bf16)
        nc.sync.dma_start(wk_sb[:], wk_d.rearrange("(c p) n -> p c n", p=P))
        nc.sync.dma_start(encT_sb[:, :, 512:1024], encT_r[:, :, 512:1024])
        wv_sb = const.tile([P, KC_KV, INNER], bf16)
        nc.sync.dma_start(wv_sb[:], wv_d.rearrange("(c p) n -> p c n", p=P))
        wo_sb = const.tile([P, AT, QD], bf16)
        nc.sync.dma_start(wo_sb[:], wo_d.rearrange("(c p) n -> p c n", p=P))
        bo_sb = const.tile([P, QD], f32)
        nc.sync.dma_start(bo_sb[:], bo_d.ap().to_broadcast((P, QD)))

        qT_sb = acts.tile([P, AT, QSH], bf16)
        kT_sb = acts.tile([P, AT, LKV], bf16)
        v_sb = acts.tile([P, NT, H * P], bf16)
        outT_sb = acts.tile([P, AT, QSH], bf16)
        vv4 = v_sb.rearrange("p t (s c) -> p t s c", c=P)

        # ---- PE warmup: dummy matmuls on zeroed scratch fill the DMA head
        # so the first real matmuls run at full clock (psD is never read)
        scratch = acts.tile([P, 512], bf16)
        nc.gpsimd.memset(scratch[:], 0.0)

        # ones column for the PE-side partition broadcast in normalize
        ones_sb = const.tile([1, P], bf16)
        nc.vector.memset(ones_sb[:], 1.0)

        def warmup(nmm):
            psD = psA.tile([P, 512], f32, tag="acc")
            for i in range(nmm):
                nc.tensor.matmul(
                    psD[:], scratch[:, 0:P], scratch[:],
                    start=(i == 0), stop=(i == nmm - 1),
                )

        # Generators yield once per emitted PE matmul so attention blocks can
        # interleave them into PE slack at a controlled rate (the per-engine
        # instruction streams execute strictly in program order).
        def gen_proj_k(a, nns=(0, 1)):
            # trailing copies are emitted BEFORE the final yield so that a
            # fully-popped generator has fully emitted its writes
            for nn in nns:
                ps = psA.tile([P, 512], f32, tag="acc")
                for c in range(KC_KV):
                    nc.tensor.matmul(
                        ps[:],
                        wk_sb[:, c, a * P:(a + 1) * P],
                        encT_sb[:, c, nn * 512:(nn + 1) * 512],
                        start=(c == 0),
                        stop=(c == KC_KV - 1),
                    )
                    if c < KC_KV - 1:
                        yield
                nc.vector.tensor_copy(kT_sb[:, a, nn * 512:(nn + 1) * 512], ps[:])
                yield

        def gen_proj_q(a, n):
            ps = psA.tile([P, 512], f32, tag="acc")
            for c in range(KC_Q):
                nc.tensor.matmul(
                    ps[:],
                    wq_sb[:, c, a * P:(a + 1) * P],
                    hsT_sb[:, c, n * 512:(n + 1) * 512],
                    start=(c == 0),
                    stop=(c == KC_Q - 1),
                )
                if c < KC_Q - 1:
                    yield
            nc.vector.tensor_copy(qT_sb[:, a, n * 512:(n + 1) * 512], ps[:])
            yield

        # v natural [kv, slots]: slot h (128 wide):
        #   h even: [v_h (0:64) | 1.0 at 64 | 0 at 65:128]   -> out rows 0:64, denom row 64
        #   h odd : [1.0 at 0 | 0 at 1:64 | v_h at 64:128]   -> out rows 64:128, denom row 0
        def v_memsets():
            for t in range(NT):
                nc.gpsimd.memset(vv4[:, t, 0::2, 64:65], 1.0)
                nc.gpsimd.memset(vv4[:, t, 1::2, 0:1], 1.0)
                nc.gpsimd.memset(vv4[:, t, 0::2, 65:P], 0.0)
                nc.gpsimd.memset(vv4[:, t, 1::2, 1:DH], 0.0)

        def gen_proj_v(t):
            ps = psA.tile([P, 512], f32, tag="acc")
            for c in range(KC_KV):
                nc.tensor.matmul(
                    ps[:],
                    encT_sb[:, c, t * P:(t + 1) * P],
                    wv_sb[:, c, :],
                    start=(c == 0),
                    stop=(c == KC_KV - 1),
                )
                if c < KC_KV - 1:
                    yield
            pv = ps.rearrange("p (s c) -> p s c", c=DH)
            nc.vector.tensor_copy(vv4[:, t, 0::2, 0:DH], pv[:, 0::2, :])
            nc.vector.tensor_copy(vv4[:, t, 1::2, DH:P], pv[:, 1::2, :])
            yield

        def gen_final(m):
            ps = psA.tile([P, 512], f32, tag="acc")
            for a in range(AT):
                nc.tensor.matmul(
                    ps[:],
                    outT_sb[:, a, m * P:(m + 1) * P],
                    wo_sb[:, a, :],
                    start=(a == 0),
                    stop=(a == AT - 1),
                )
                if a < AT - 1:
                    yield
            ob = outp.tile([P, QD], f32)
            nc.vector.tensor_add(ob[:], ps[:], bo_sb[:])
            nc.sync.dma_start(out_d[m * P:(m + 1) * P, :], ob[:])
            yield

        # final projection split for the tail m-tiles: partA (heads 0-1)
        # accumulates into an SBUF staging tile during earlier blocks; partB
        # (heads 2-3) only trails the last attention block
        facc = acts.tile([P, QSH // P, QD], f32)

        def gen_final_a(m):
            ps = psA.tile([P, 512], f32, tag="acc")
            for a in (0, 1):
                nc.tensor.matmul(
                    ps[:],
                    outT_sb[:, a, m * P:(m + 1) * P],
                    wo_sb[:, a, :],
                    start=(a == 0),
                    stop=(a == 1),
                )
                if a == 0:
                    yield
            nc.vector.tensor_add(facc[:, m, :], ps[:], bo_sb[:])
            yield

        def gen_final_b(m):
            ps = psA.tile([P, 512], f32, tag="acc")
            for a in (2, 3):
                nc.tensor.matmul(
                    ps[:],
                    outT_sb[:, a, m * P:(m + 1) * P],
                    wo_sb[:, a, :],
                    start=(a == 2),
                    stop=(a == 3),
                )
                if a == 2:
                    yield
            ob = outp.tile([P, QD], f32)
            nc.vector.tensor_add(ob[:], ps[:], facc[:, m, :])
            nc.sync.dma_start(out_d[m * P:(m + 1) * P, :], ob[:])
            yield

        def gen_chain(*gens):
            for g in gens:
                yield from g

        def run_gen(g):
            for _ in g:
                pass

        def attn(hp, n, extras=None, pre_pop=0, per_iter=0, prev_tail=None,
                 drain=True, act_copy_norm=False):
            """Emit one attention block.  Returns a closure that emits the
            block's last two AV matmuls + normalize; the caller passes it to
            the NEXT block so those trail instructions interleave with the
            next block's leading scores (removes the block-boundary bubble).
            """
            if extras is None:
                extras = iter(())

            def pop(k):
                for _ in range(k):
                    if next(extras, StopIteration) is StopIteration:
                        break

            av0 = psA.tile([P, 512], f32, tag="acc")
            av1 = psA.tile([P, 512], f32, tag="acc")
            av = (av0, av1)
            exs = []

            def s_(t):
                ss = psS.tile([P, 1024], f32)
                for i in range(2):
                    pr = slice(i * 64, (i + 1) * 64)
                    nc.tensor.matmul(
                        ss[:, i * 512:(i + 1) * 512],
                        kT_sb[pr, hp, t * P:(t + 1) * P],
                        qT_sb[pr, hp, n * 512:(n + 1) * 512],
                        start=True,
                        stop=True,
                    )
                ex = expp.tile([P, 1024], bf16)
                nc.scalar.activation(ex[:], ss[:], Exp, scale=SCALE)
                exs.append(ex)

            def A_(t):
                for i in range(2):
                    s = 2 * hp + i
                    nc.tensor.matmul(
                        av[i][:],
                        v_sb[:, t, s * P:(s + 1) * P],
                        exs[t][:, i * 512:(i + 1) * 512],
                        start=(t == 0),
                        stop=(t == NT - 1),
                    )

            s_(0)
            s_(1)
            pop(pre_pop)
            if prev_tail is not None:
                prev_tail()
            for t in range(2, NT):
                s_(t)
                A_(t - 2)
                pop(per_iter)
            if drain:  # drain leftovers so every generator completes
                for _ in extras:
                    pass

            def tail():
                A_(NT - 2)
                A_(NT - 1)
                # partition broadcast of 1/denom via a PE ones-column matmul
                # (GpSimd partition_broadcast proved flaky on HW; the DMA
                # round-trip costs ~4us per block).  The reciprocal lands on
                # partition 0 in bf16, ones.T @ recip fills a PSUM tile,
                # which is copied to SBUF for the multiply (ScalarE for the
                # last block where it is idle, DVE elsewhere).
                for i in range(2):
                    drow = 64 if i == 0 else 0
                    dst = slice(0, 64) if i == 0 else slice(64, 128)
                    rc = small.tile([1, 512], bf16, tag="rc")
                    with nc.allow_low_precision(
                        reason="softmax denom reciprocal, bf16 suffices"
                    ):
                        nc.vector.reciprocal(
                            rc[0:1, :], av[i][drow:drow + 1, :]
                        )
                    rcps = psA.tile([P, 512], f32, tag="acc")
                    nc.tensor.matmul(
                        rcps[:], ones_sb[0:1, :], rc[0:1, :],
                        start=True, stop=True,
                    )
                    rcb = small.tile([P, 512], f32, tag="rcb")
                    if act_copy_norm:
                        nc.scalar.copy(rcb[:], rcps[:])
                    else:
                        nc.vector.tensor_copy(rcb[:], rcps[:])
                    nc.vector.tensor_mul(
                        outT_sb[dst, hp, n * 512:(n + 1) * 512],
                        av[i][dst, :],
                        rcb[dst, :],
                    )

            return tail

        # ---- emission = per-engine execution order.  Warmup dummies bridge
        # the DMA head up to qT(0,0); kT(0) kv-half 0 slots into the gap as
        # soon as its DMA lands; everything else (v, kT second half, later
        # k/q projections, finals) interleaves into attention-block PE slack.
        v_memsets()
        warmup(CFG["W1"])
        for a in range(AT):
            run_gen(gen_proj_q(a, 0))
        run_gen(gen_proj_q(0, 1))
        run_gen(gen_proj_k(0))
        tail = attn(
            0, 0,
            extras=gen_chain(
                *[gen_proj_v(t) for t in range(NT)],
                gen_proj_k(1),
            ),
            pre_pop=CFG["B0_PRE"], per_iter=CFG["B0_PER"],
        )
        tail = attn(1, 0, extras=gen_chain(gen_proj_k(2), gen_proj_q(1, 1)),
                    pre_pop=CFG["BK_PRE"], per_iter=CFG["BK_PER"],
                    prev_tail=tail)
        tail = attn(2, 0, extras=gen_chain(gen_proj_k(3), gen_proj_q(2, 1)),
                    pre_pop=CFG["BK_PRE"], per_iter=CFG["BK_PER"],
                    prev_tail=tail)
        tail = attn(3, 0, extras=gen_proj_q(3, 1), pre_pop=0, per_iter=1,
                    prev_tail=tail)
        f01 = gen_chain(gen_final(0), gen_final(1))
        tail = attn(0, 1, extras=f01, pre_pop=0, per_iter=1,
                    prev_tail=tail, drain=False)
        tail = attn(1, 1, extras=gen_chain(f01, gen_final(2)),
                    pre_pop=0, per_iter=1, prev_tail=tail)
        tail = attn(2, 1,
                    extras=gen_chain(gen_final(3), gen_final_a(4)),
                    pre_pop=0, per_iter=CFG["B21_PER"], prev_tail=tail)
        tail = attn(3, 1, act_copy_norm=True,
                    extras=gen_chain(gen_final_a(5), gen_final_a(6),
                                     gen_final_a(7)),
                    pre_pop=0, per_iter=CFG["B31_PER"], prev_tail=tail)
        tail()
        for m in range(AT, QSH // P):
            run_gen(gen_final_b(m))

    nc.finalize()
    return nc


def _get_nc():
    if "nc" not in _CACHE:
        _CACHE["nc"] = _build_nc()
    return _CACHE["nc"]


def make_in_maps(hidden_states, encoder_hidden_states, Wq, Wk, Wv, Wo, bo):
    import ml_dtypes

    bf16 = ml_dtypes.bfloat16
    hs = np.asarray(hidden_states, dtype=np.float32)
    enc = np.asarray(encoder_hidden_states, dtype=np.float32)
    wq = np.ascontiguousarray(np.asarray(Wq, dtype=np.float32)).astype(bf16)
    wk = np.ascontiguousarray(np.asarray(Wk, dtype=np.float32)).astype(bf16)
    wv = np.ascontiguousarray(np.asarray(Wv, dtype=np.float32)).astype(bf16)
    wo = np.ascontiguousarray(np.asarray(Wo, dtype=np.float32)).astype(bf16)
    bo_ = np.ascontiguousarray(np.asarray(bo, dtype=np.float32)).reshape(1, QD)
    encT = [np.ascontiguousarray(enc[b].T).astype(bf16) for b in range(B)]
    in_maps = []
    for c in range(NCORES):
        b, s = divmod(c, 4)
        hsT = np.ascontiguousarray(hs[b, s * QSH:(s + 1) * QSH, :].T).astype(bf16)
        in_maps.append(
            dict(hsT=hsT, encT=encT[b], wq=wq, wk=wk, wv=wv, wo=wo, bo=bo_)
        )
    return in_maps


def kernel(hidden_states, encoder_hidden_states, Wq, Wk, Wv, Wo, bo):
    global LAST_RESULTS
    from concourse.bass_utils import run_bass_kernel_spmd

    nc = _get_nc()
    in_maps = make_in_maps(
        hidden_states, encoder_hidden_states, Wq, Wk, Wv, Wo, bo
    )
    res = run_bass_kernel_spmd(nc, in_maps, core_ids=list(range(NCORES)))
    LAST_RESULTS = res
    out = np.empty((B, LQ, QD), dtype=np.float32)
    for c in range(NCORES):
        b, s = divmod(c, 4)
        out[b, s * QSH:(s + 1) * QSH, :] = res.results[c]["out"]
    return out



# revision 23
# speedup vs baseline: 1.0598x; 1.0598x over previous
"""CrossAttention kernel for 8 TRN2 NeuronCores.

Reference computation (B=2, Lq=4096, Lkv=1024, query_dim=512, cross_dim=768,
heads=8, dim_head=64, inner=512):
    q = hs @ Wq; k = enc @ Wk; v = enc @ Wv          (per batch)
    attn = softmax(q_h @ k_h^T * scale) per head
    out = concat_h(attn @ v_h) @ Wo + bo

Sharding: 8 cores = 2 batches x 4 query-slices of 1024 queries.  Each core
computes its full slice of the output (all heads), so outputs are disjoint
and no collective is needed.

Per-core dataflow (all matmuls f16 operands, fp32 PSUM accumulate):
  - host passes hs-slice and encoder transposed (hsT [512,1024], encT
    [768,1024]) already cast to f16, weights in f16
  - qT = Wq^T-weighted hsT   -> [inner=512, q=1024]  (heads along partitions)
  - kT likewise              -> [inner=512, kv=1024]
  - v natural                -> [kv=1024, slots]  slot h = 128 cols holding
        v_h (64) + a ones column + zero padding, arranged so the AV matmul
        output lands partition-aligned with head h's rows of outT and the
        softmax denominator (sum_kv exp) falls out of the same matmul.
  - scoresT_h = k_h qT_h     -> [kv, q] (kv on partitions; head pairs use
        base-partition row tiling of the 128x128 PE array)
  - expT = exp(scale * scoresT) on ScalarE, f16 out (no max-subtraction:
        |scaled scores| < ~3)
  - outT_unnorm_h = v_slot^T @ expT accumulated over kv chunks (PSUM),
        one row of which is the softmax denominator
  - normalize: reciprocal (DVE) + PE ones-column broadcast matmul + multiply
  - final = outT^T @ Wo + bo -> [1024, 512], DMA out per 128-row tile

Program order is pipelined for the Tile scheduler: k/v/q projections are
emitted ahead of the attention blocks that consume them, exp(t) is emitted
before AV(t-1) so the PE never waits in-order on ScalarE, and the final
projection m-tiles are interleaved between the last attention blocks.
"""

import sys

if "/opt/trn_rl_repo" not in sys.path:
    sys.path.insert(0, "/opt/trn_rl_repo")

import numpy as np

B, LQ, LKV = 2, 4096, 1024
QD, CD = 512, 768
H, DH = 8, 64
INNER = H * DH  # 512
SCALE = DH ** -0.5
NCORES = 8
WSCALE = 8.0  # host-side pre-scale on Wq and Wk (fp8 range centering)
QSH = LQ // 4  # 1024 queries per core
P = 128

_CACHE: dict = {}
LAST_RESULTS = None  # test harness introspection (exec_time_ns etc.)

# schedule-tuning knobs (sweepable from bench tooling)
CFG = {
    "W1": 12,       # warmup matmuls bridging the input-DMA head
    "B0_PRE": 8,    # block (0,0) pre-loop extra pops
    "B0_PER": 9,    # block (0,0) per-iter extra pops
    "BK_PRE": 2,    # later n=0 blocks pre-loop pops
    "BK_PER": 2,    # later n=0 blocks per-iter pops
    "B21_PER": 1,   # block (2,1) per-iter pops
    "B31_PER": 1,   # block (3,1) per-iter pops
}


def _build_nc():
    from contextlib import ExitStack

    import concourse.bass as bass
    import concourse.tile as tile
    from concourse import bacc, mybir

    f32 = mybir.dt.float32
    f16 = mybir.dt.float16
    f8 = mybir.dt.float8e4
    DR = mybir.MatmulPerfMode.DoubleRow
    Exp = mybir.ActivationFunctionType.Exp

    nc = bacc.Bacc(trn_type="TRN2")

    hsT_d = nc.declare_dram_parameter("hsT", [QD, QSH], f16, isOutput=False)
    encT_d = nc.declare_dram_parameter("encT", [CD, LKV], f16, isOutput=False)
    wq_d = nc.declare_dram_parameter("wq", [QD, INNER], f16, isOutput=False)
    wk_d = nc.declare_dram_parameter("wk", [CD, INNER], f16, isOutput=False)
    wv_d = nc.declare_dram_parameter("wv", [CD, INNER], f16, isOutput=False)
    wo_d = nc.declare_dram_parameter("wo", [INNER, QD], f16, isOutput=False)
    bo_d = nc.declare_dram_parameter("bo", [1, QD], f32, isOutput=False)
    out_d = nc.declare_dram_parameter("out", [QSH, QD], f32, isOutput=True)

    KC_Q = QD // P   # 4 contraction chunks for q projection
    KC_KV = CD // P  # 6 for k/v projections
    AT = INNER // P  # 4 inner tiles (2 heads each)
    NT = LKV // P    # 8 kv chunks
    QN = QSH // 512  # 2 q slices of 512

    with ExitStack() as ctx:
        tc = ctx.enter_context(tile.TileContext(nc))
        const = ctx.enter_context(tc.tile_pool(name="const", bufs=1))
        acts = ctx.enter_context(tc.tile_pool(name="acts", bufs=1))
        expp = ctx.enter_context(tc.tile_pool(name="expp", bufs=4))
        outp = ctx.enter_context(tc.tile_pool(name="outp", bufs=4))
        small = ctx.enter_context(tc.tile_pool(name="small", bufs=6))
        psA = ctx.enter_context(tc.tile_pool(name="psA", bufs=4, space="PSUM"))
        psS = ctx.enter_context(tc.tile_pool(name="psS", bufs=2, space="PSUM"))
        drp = ctx.enter_context(tc.tile_pool(name="drp", bufs=4, space="DRAM"))

        # ---- input DMA, ordered by first use: the q projections (hsT+wq)
        # run during the PE warmup window, then kT (encT+wk), then v (wv);
        # the second encT half only gates scores t>=4 of the first block
        hsT_sb = acts.tile([P, KC_Q, QSH], f16)
        nc.sync.dma_start(hsT_sb[:], hsT_d.rearrange("(c p) n -> p c n", p=P))
        wq_sb = const.tile([P, KC_Q, INNER], f16)
        nc.sync.dma_start(wq_sb[:], wq_d.rearrange("(c p) n -> p c n", p=P))
        encT_sb = acts.tile([P, KC_KV, LKV], f16)
        encT_r = encT_d.rearrange("(c p) n -> p c n", p=P)
        nc.sync.dma_start(encT_sb[:, :, 0:512], encT_r[:, :, 0:512])
        wk_sb = const.tile([P, KC_KV, INNER], f16)
        nc.sync.dma_start(wk_sb[:], wk_d.rearrange("(c p) n -> p c n", p=P))
        nc.sync.dma_start(encT_sb[:, :, 512:1024], encT_r[:, :, 512:1024])
        wv_sb = const.tile([P, KC_KV, INNER], f16)
        nc.sync.dma_start(wv_sb[:], wv_d.rearrange("(c p) n -> p c n", p=P))
        wo_sb = const.tile([P, AT, QD], f16)
        nc.sync.dma_start(wo_sb[:], wo_d.rearrange("(c p) n -> p c n", p=P))
        bo_sb = const.tile([P, QD], f32)
        nc.sync.dma_start(bo_sb[:], bo_d.ap().to_broadcast((P, QD)))

        qT8 = acts.tile([P, AT, 2, QSH], f8)       # planes: (hi, lo)
        kT8 = acts.tile([P, AT, LKV], f8)          # single plane, broadcast in DR
        v_sb = acts.tile([P, NT, H * P], f16)
        outT_sb = acts.tile([P, AT, QSH], f16)
        vv4 = v_sb.rearrange("p t (s c) -> p t s c", c=P)

        # ---- PE warmup: dummy matmuls on zeroed scratch fill the DMA head
        # so the first real matmuls run at full clock (psD is never read)
        scratch = acts.tile([P, 512], f16)
        nc.gpsimd.memset(scratch[:], 0.0)

        # ones column for the PE-side partition broadcast in normalize
        ones_sb = const.tile([1, P], f16)
        nc.vector.memset(ones_sb[:], 1.0)

        def warmup(nmm):
            psD = psA.tile([P, 512], f32, tag="acc")
            for i in range(nmm):
                nc.tensor.matmul(
                    psD[:], scratch[:, 0:P], scratch[:],
                    start=(i == 0), stop=(i == nmm - 1),
                )

        # Generators yield once per emitted PE matmul so attention blocks can
        # interleave them into PE slack at a controlled rate (the per-engine
        # instruction streams execute strictly in program order).
        def gen_proj_k(a, nns=(0, 1)):
            # trailing copies are emitted BEFORE the final yield so that a
            # fully-popped generator has fully emitted its writes
            for nn in nns:
                ps = psA.tile([P, 512], f32, tag="acc")
                for c in range(KC_KV):
                    nc.tensor.matmul(
                        ps[:],
                        wk_sb[:, c, a * P:(a + 1) * P],
                        encT_sb[:, c, nn * 512:(nn + 1) * 512],
                        start=(c == 0),
                        stop=(c == KC_KV - 1),
                    )
                    if c < KC_KV - 1:
                        yield
                with nc.allow_low_precision(reason="k stored fp8 for DR scores"):
                    nc.vector.tensor_copy(
                        kT8[:, a, nn * 512:(nn + 1) * 512], ps[:]
                    )
                yield

        def gen_proj_q(a, n):
            ps = psA.tile([P, 512], f32, tag="acc")
            for c in range(KC_Q):
                nc.tensor.matmul(
                    ps[:],
                    wq_sb[:, c, a * P:(a + 1) * P],
                    hsT_sb[:, c, n * 512:(n + 1) * 512],
                    start=(c == 0),
                    stop=(c == KC_Q - 1),
                )
                if c < KC_Q - 1:
                    yield
            sl = slice(n * 512, (n + 1) * 512)
            with nc.allow_low_precision(reason="q stored as fp8 hi/lo pair"):
                nc.vector.tensor_copy(qT8[:, a, 0, sl], ps[:])
                nc.vector.tensor_sub(qT8[:, a, 1, sl], ps[:], qT8[:, a, 0, sl])
            yield

        # v natural [kv, slots]: slot h (128 wide):
        #   h even: [v_h (0:64) | 1.0 at 64 | 0 at 65:128]   -> out rows 0:64, denom row 64
        #   h odd : [1.0 at 0 | 0 at 1:64 | v_h at 64:128]   -> out rows 64:128, denom row 0
        def v_memsets():
            nc.gpsimd.memset(vv4[:, :, 0::2, 64:65], 1.0)
            nc.gpsimd.memset(vv4[:, :, 1::2, 0:1], 1.0)
            nc.gpsimd.memset(vv4[:, :, 0::2, 65:P], 0.0)
            nc.gpsimd.memset(vv4[:, :, 1::2, 1:DH], 0.0)

        def gen_proj_v(t):
            ps = psA.tile([P, 512], f32, tag="acc")
            for c in range(KC_KV):
                nc.tensor.matmul(
                    ps[:],
                    encT_sb[:, c, t * P:(t + 1) * P],
                    wv_sb[:, c, :],
                    start=(c == 0),
                    stop=(c == KC_KV - 1),
                )
                if c < KC_KV - 1:
                    yield
            pv = ps.rearrange("p (s c) -> p s c", c=DH)
            nc.vector.tensor_copy(vv4[:, t, 0::2, 0:DH], pv[:, 0::2, :])
            nc.vector.tensor_copy(vv4[:, t, 1::2, DH:P], pv[:, 1::2, :])
            yield

        def gen_final(m):
            ps = psA.tile([P, 512], f32, tag="acc")
            for a in range(AT):
                nc.tensor.matmul(
                    ps[:],
                    outT_sb[:, a, m * P:(m + 1) * P],
                    wo_sb[:, a, :],
                    start=(a == 0),
                    stop=(a == AT - 1),
                )
                if a < AT - 1:
                    yield
            ob = outp.tile([P, QD], f32)
            nc.vector.tensor_add(ob[:], ps[:], bo_sb[:])
            nc.sync.dma_start(out_d[m * P:(m + 1) * P, :], ob[:])
            yield

        # final projection split for the tail m-tiles: partA (heads 0-1)
        # accumulates into an SBUF staging tile during earlier blocks; partB
        # (heads 2-3) only trails the last attention block
        facc = acts.tile([P, QSH // P, QD], f32)

        def gen_final_a(m):
            ps = psA.tile([P, 512], f32, tag="acc")
            for a in (0, 1):
                nc.tensor.matmul(
                    ps[:],
                    outT_sb[:, a, m * P:(m + 1) * P],
                    wo_sb[:, a, :],
                    start=(a == 0),
                    stop=(a == 1),
                )
                if a == 0:
                    yield
            nc.vector.tensor_add(facc[:, m, :], ps[:], bo_sb[:])
            yield

        def gen_final_b(m):
            ps = psA.tile([P, 512], f32, tag="acc")
            for a in (2, 3):
                nc.tensor.matmul(
                    ps[:],
                    outT_sb[:, a, m * P:(m + 1) * P],
                    wo_sb[:, a, :],
                    start=(a == 2),
                    stop=(a == 3),
                )
                if a == 2:
                    yield
            ob = outp.tile([P, QD], f32)
            nc.vector.tensor_add(ob[:], ps[:], facc[:, m, :])
            nc.sync.dma_start(out_d[m * P:(m + 1) * P, :], ob[:])
            yield

        def gen_chain(*gens):
            for g in gens:
                yield from g

        def run_gen(g):
            for _ in g:
                pass

        def attn(hp, n, extras=None, pre_pop=0, per_iter=0, prev_tail=None,
                 drain=True, act_copy_norm=False):
            """Emit one attention block.  Returns a closure that emits the
            block's last two AV matmuls + normalize; the caller passes it to
            the NEXT block so those trail instructions interleave with the
            next block's leading scores (removes the block-boundary bubble).
            """
            if extras is None:
                extras = iter(())

            def pop(k):
                for _ in range(k):
                    if next(extras, StopIteration) is StopIteration:
                        break

            av0 = psA.tile([P, 512], f32, tag="acc")
            av1 = psA.tile([P, 512], f32, tag="acc")
            av = (av0, av1)
            exs = []

            def s_(t):
                ss = psS.tile([P, 1024], f32)
                for i in range(2):
                    pr = slice(i * 64, (i + 1) * 64)
                    nc.tensor.matmul(
                        ss[:, i * 512:(i + 1) * 512],
                        kT8[pr, hp, t * P:(t + 1) * P]
                        .unsqueeze(1).broadcast_to((64, 2, P)),
                        qT8[pr, hp, :, n * 512:(n + 1) * 512],
                        start=True,
                        stop=True,
                        perf_mode=DR,
                    )
                ex = expp.tile([P, 1024], f16)
                nc.scalar.activation(
                    ex[:], ss[:], Exp, scale=SCALE / (WSCALE * WSCALE)
                )
                exs.append(ex)

            def A_(t):
                for i in range(2):
                    s = 2 * hp + i
                    nc.tensor.matmul(
                        av[i][:],
                        v_sb[:, t, s * P:(s + 1) * P],
                        exs[t][:, i * 512:(i + 1) * 512],
                        start=(t == 0),
                        stop=(t == NT - 1),
                    )

            s_(0)
            s_(1)
            pop(pre_pop)
            if prev_tail is not None:
                prev_tail()
            for t in range(2, NT):
                s_(t)
                A_(t - 2)
                pop(per_iter)
            if drain:  # drain leftovers so every generator completes
                for _ in extras:
                    pass

            def tail():
                A_(NT - 2)
                A_(NT - 1)
                # partition broadcast of 1/denom via a PE ones-column matmul
                # (GpSimd partition_broadcast proved flaky on HW; the DMA
                # round-trip costs ~4us per block).  The reciprocal lands on
                # partition 0 in f16, ones.T @ recip fills a PSUM tile,
                # which is copied to SBUF for the multiply (ScalarE for the
                # last block where it is idle, DVE elsewhere).
                for i in range(2):
                    drow = 64 if i == 0 else 0
                    dst = slice(0, 64) if i == 0 else slice(64, 128)
                    rc = small.tile([1, 512], f16, tag="rc")
                    with nc.allow_low_precision(
                        reason="softmax denom reciprocal, f16 suffices"
                    ):
                        nc.vector.reciprocal(
                            rc[0:1, :], av[i][drow:drow + 1, :]
                        )
                    rcps = psA.tile([P, 512], f32, tag="acc")
                    nc.tensor.matmul(
                        rcps[:], ones_sb[0:1, :], rc[0:1, :],
                        start=True, stop=True,
                    )
                    rcb = small.tile([P, 512], f32, tag="rcb")
                    if act_copy_norm:
                        nc.scalar.copy(rcb[:], rcps[:])
                    else:
                        nc.vector.tensor_copy(rcb[:], rcps[:])
                    nc.vector.tensor_mul(
                        outT_sb[dst, hp, n * 512:(n + 1) * 512],
                        av[i][dst, :],
                        rcb[dst, :],
                    )

            return tail

        # ---- emission = per-engine execution order.  Warmup dummies bridge
        # the DMA head up to qT(0,0); kT(0) kv-half 0 slots into the gap as
        # soon as its DMA lands; everything else (v, kT second half, later
        # k/q projections, finals) interleaves into attention-block PE slack.
        v_memsets()
        warmup(CFG["W1"])
        for a in range(AT):
            run_gen(gen_proj_q(a, 0))
        run_gen(gen_proj_q(0, 1))
        run_gen(gen_proj_k(0))
        tail = attn(
            0, 0,
            extras=gen_chain(
                *[gen_proj_v(t) for t in range(NT)],
                gen_proj_k(1),
            ),
            pre_pop=CFG["B0_PRE"], per_iter=CFG["B0_PER"],
        )
        tail = attn(1, 0, extras=gen_chain(gen_proj_k(2), gen_proj_q(1, 1)),
                    pre_pop=CFG["BK_PRE"], per_iter=CFG["BK_PER"],
                    prev_tail=tail)
        tail = attn(2, 0, extras=gen_chain(gen_proj_k(3), gen_proj_q(2, 1)),
                    pre_pop=CFG["BK_PRE"], per_iter=CFG["BK_PER"],
                    prev_tail=tail)
        tail = attn(3, 0, extras=gen_proj_q(3, 1), pre_pop=0, per_iter=1,
                    prev_tail=tail)
        f01 = gen_chain(gen_final(0), gen_final(1))
        tail = attn(0, 1, extras=f01, pre_pop=0, per_iter=1,
                    prev_tail=tail, drain=False)
        tail = attn(1, 1, extras=gen_chain(f01, gen_final(2)),
                    pre_pop=0, per_iter=1, prev_tail=tail)
        tail = attn(2, 1,
                    extras=gen_chain(gen_final(3), gen_final_a(4)),
                    pre_pop=0, per_iter=CFG["B21_PER"], prev_tail=tail)
        tail = attn(3, 1, act_copy_norm=True,
                    extras=gen_chain(gen_final_a(5), gen_final_a(6),
                                     gen_final_a(7)),
                    pre_pop=0, per_iter=CFG["B31_PER"], prev_tail=tail)
        tail()
        for m in range(AT, QSH // P):
            run_gen(gen_final_b(m))

    nc.finalize()
    return nc


def _get_nc():
    if "nc" not in _CACHE:
        _CACHE["nc"] = _build_nc()
    return _CACHE["nc"]


def make_in_maps(hidden_states, encoder_hidden_states, Wq, Wk, Wv, Wo, bo):
    import ml_dtypes

    f16 = ml_dtypes.bfloat16
    hs = np.asarray(hidden_states, dtype=np.float32)
    enc = np.asarray(encoder_hidden_states, dtype=np.float32)
    wq = np.ascontiguousarray(np.asarray(Wq, dtype=np.float32)).astype(f16)
    wk = np.ascontiguousarray(np.asarray(Wk, dtype=np.float32)).astype(f16)
    wv = np.ascontiguousarray(np.asarray(Wv, dtype=np.float32)).astype(f16)
    wo = np.ascontiguousarray(np.asarray(Wo, dtype=np.float32)).astype(f16)
    bo_ = np.ascontiguousarray(np.asarray(bo, dtype=np.float32)).reshape(1, QD)
    encT = [np.ascontiguousarray(enc[b].T).astype(f16) for b in range(B)]
    in_maps = []
    for c in range(NCORES):
        b, s = divmod(c, 4)
        hsT = np.ascontiguousarray(hs[b, s * QSH:(s + 1) * QSH, :].T).astype(f16)
        in_maps.append(
            dict(hsT=hsT, encT=encT[b], wq=wq, wk=wk, wv=wv, wo=wo, bo=bo_)
        )
    return in_maps


def kernel(hidden_states, encoder_hidden_states, Wq, Wk, Wv, Wo, bo):
    global LAST_RESULTS
    from concourse.bass_utils import run_bass_kernel_spmd

    nc = _get_nc()
    in_maps = make_in_maps(
        hidden_states, encoder_hidden_states, Wq, Wk, Wv, Wo, bo
    )
    res = run_bass_kernel_spmd(nc, in_maps, core_ids=list(range(NCORES)))
    LAST_RESULTS = res
    out = np.empty((B, LQ, QD), dtype=np.float32)
    for c in range(NCORES):
        b, s = divmod(c, 4)
        out[b, s * QSH:(s + 1) * QSH, :] = res.results[c]["out"]
    return out

